# revision 35
# baseline (speedup 1.0000x reference)
"""EMA head kernel for Trainium2 (Bass/Tile), 8 NeuronCores.

Problem: alpha = clip(sigmoid(MLP(feat)), 0.01, 0.99) per (t, b);
         y[0] = r[0]; y[t] = (1-alpha[t])*y[t-1] + alpha[t]*r[t].

Sharding: time dim T=4096 split into 8 slabs of 512 (all B=256 per core),
and each slab further into 4 chunks of 128 on-chip.  Each core computes,
per 128-t chunk, the local affine-scan pieces
    z[t] = A[t]*z[t-1] + Bv[t]   (z[-1] = 0),   A = 1-alpha, Bv = alpha*r
    P[t] = A[t]*P[t-1]           (P[-1] = 1)
and the host stitches all 32 chunks with  y = z + P * carry,
carry' = y[-1].  carry_0 = r[0] reproduces y[0] = r[0] exactly.

On-chip layout (DMA-bandwidth oriented): feat tiles keep TIME on the
partition dim: tile = feat[t0:t0+128, b0:b0+32, :] -> [128 t, 32 b, 128 f]
so each partition line is ONE contiguous 16 KB DRAM read (512 B chunks
when batch is on partitions starve the SDMA engines).  fp32->fp16 cast
happens in the SWDGE DMA.  Per b: PE transpose [t, f] -> [f, t], matmul
lhsT=[f,t] rhs=W1 -> h [t, 16] into a PSUM bank of 32 slots, then
+b1/relu/*W2/reduce -> alpha_pre [t, b], sigmoid+clip -> alpha [t, b],
Bv = alpha*r (r also loads [t, b] natively).  Per chunk, 4 small PE
transposes flip alpha/Bv to [b, t] and DVE tensor_tensor_scan runs along
the free (t=128) dim for z and P.  Param broadcasts (b1/w2/b2) are built
on-chip with K=1 outer-product matmuls -- DMA DRE replication is
pathologically slow (~7.7us per 4 KB packet) and clogs the SWDGE queue.
"""

import numpy as np

T, B, FEAT, HID = 4096, 256, 128, 16
NCORES = 8
TLOC = T // NCORES  # 512
NTG = TLOC // 128   # 4 t-chunks of 128 (the partition dim)
BQ = 32             # b-chunk per feat tile (16 KB/partition contiguous)
NBQ = B // BQ       # 8
NH = 2              # b halves of 128 for the scan layout

_CACHE = {}


def _build_program():
    import concourse.bacc as bacc
    import concourse.bass as bass
    import concourse.tile as tile
    from concourse import mybir
    from concourse.masks import make_identity

    fp32 = mybir.dt.float32
    fp16 = mybir.dt.float16
    AF = mybir.ActivationFunctionType
    OP = mybir.AluOpType

    nc = bacc.Bacc("TRN2", target_bir_lowering=False, debug=False,
                   num_devices=NCORES)

    feat_d = nc.dram_tensor("feat", [TLOC, B, FEAT], fp32, kind="ExternalInput")
    r_d = nc.dram_tensor("r", [TLOC, B], fp32, kind="ExternalInput")
    w1_d = nc.dram_tensor("w1", [FEAT, HID], fp32, kind="ExternalInput")
    b1_d = nc.dram_tensor("b1", [HID], fp32, kind="ExternalInput")
    w2_d = nc.dram_tensor("w2", [HID], fp32, kind="ExternalInput")
    b2_d = nc.dram_tensor("b2", [1], fp32, kind="ExternalInput")
    # packed z/p output: [b 128, {z,p}, half, t] -> one DMA, 4KB lines
    zp_d = nc.dram_tensor("zp", [128, 2, NH, TLOC], fp16,
                          kind="ExternalOutput")

    with tile.TileContext(nc) as tc:
        with (
            tc.tile_pool(name="singles", bufs=1) as singles,
            tc.tile_pool(name="featin", bufs=12) as featin,
            tc.tile_pool(name="ftps", bufs=4, space="PSUM") as ftps,
            tc.tile_pool(name="hps", bufs=2, space="PSUM") as hps,
            tc.tile_pool(name="atps", bufs=2, space="PSUM") as atps,
            tc.tile_pool(name="ftsb", bufs=3) as ftsb,
            tc.tile_pool(name="hwork", bufs=2) as hwork,
        ):
            # first feat tile DMAs go out before anything else queues on
            # the SWDGE ring
            NPEEL = 2
            fin_pre = []
            for i in range(NPEEL):
                fp = featin.tile([128, BQ, FEAT], fp16, tag="fin")
                nc.gpsimd.dma_start(fp, feat_d[0:128, i * BQ:(i + 1) * BQ, :])
                fin_pre.append(fp)

            # ---- tiny param rows (keep the SWDGE queue free for feat) ----
            w1_f32 = singles.tile([128, HID], fp32)
            nc.sync.dma_start(w1_f32, w1_d[:, :])
            b1row = singles.tile([1, BQ, HID], fp16)
            nc.gpsimd.dma_start(
                b1row, bass.AP(b1_d, 0, [[0, 1], [0, BQ], [1, HID]]))
            w2row = singles.tile([1, BQ, HID], fp16)
            nc.gpsimd.dma_start(
                w2row, bass.AP(w2_d, 0, [[0, 1], [0, BQ], [1, HID]]))
            b2row = singles.tile([1, 1], fp16)
            nc.gpsimd.dma_start(b2row, bass.AP(b2_d, 0, [[0, 1], [1, 1]]))

            # r loads natively as [t, b]: contiguous 1 KB lines, no transpose
            rsb = [singles.tile([128, B], fp32, tag=f"r{g}", name=f"r{g}")
                   for g in range(NTG)]
            for g in range(NTG):
                nc.sync.dma_start(rsb[g], r_d[g * 128:(g + 1) * 128, :])

            # ---------------- constants ----------------
            ident = singles.tile([128, 128], fp16)
            make_identity(nc, ident)
            ident32 = singles.tile([128, 128], fp32)
            make_identity(nc, ident32)
            ones_col = singles.tile([1, 128], fp16)
            nc.vector.memset(ones_col, 1.0)
            ones_sb = singles.tile([128, 128], fp32)
            nc.vector.memset(ones_sb, 1.0)

            w1_sb = singles.tile([128, HID], fp16)
            nc.vector.tensor_copy(w1_sb, w1_f32)

            # broadcast w2/b2 across partitions: ones_col.T @ row
            # (b1 is injected into PSUM per tile via a K=1 matmul instead)
            w2rep = singles.tile([128, BQ, HID], fp32)
            b2col = singles.tile([128, 1], fp32)
            reps2 = atps.tile([128, BQ * HID], fp32, tag="aps")
            nc.tensor.matmul(reps2, ones_col[:, :],
                             w2row[:, :, :].rearrange("p a b -> p (a b)"))
            nc.vector.tensor_copy(
                w2rep[:, :, :].rearrange("p a b -> p (a b)"), reps2)
            reps3 = atps.tile([128, BQ * HID], fp32, tag="aps")
            nc.tensor.matmul(reps3[:, 0:1], ones_col[:, :], b2row[:, :])
            nc.vector.tensor_copy(b2col, reps3[:, 0:1])

            # packed z/P accumulator, [b, {z,p}, half, t]; one end DMA
            zpacc = singles.tile([128, 2, NH, TLOC], fp16, name="zpacc")

            copy_parity = 0
            # ---------------- main feat pipeline ----------------
            for g in range(NTG):
                apre = singles.tile([128, B], fp32, tag=f"apre{g}",
                                    name=f"apre{g}")
                for bq in range(NBQ):
                    if g == 0 and bq < NPEEL:
                        fin = fin_pre[bq]
                    else:
                        fin = featin.tile([128, BQ, FEAT], fp16, tag="fin")
                        nc.gpsimd.dma_start(
                            fin, feat_d[g * 128:(g + 1) * 128,
                                        bq * BQ:(bq + 1) * BQ, :])
                    hbank = hps.tile([128, BQ, HID], fp32, tag="hb")
                    # seed the whole bank with b1 (K=1 outer product);
                    # the per-slot matmuls then accumulate on top
                    nc.tensor.matmul(
                        hbank[:, :, :].rearrange("p a b -> p (a b)"),
                        ones_col[:, :],
                        b1row[:, :, :].rearrange("p a b -> p (a b)"),
                        start=True, stop=False)
                    for q in range(0, BQ, 4):
                        ftp = ftps.tile([128, 4, 128], fp16, tag="ftp")
                        for s in range(4):
                            nc.tensor.transpose(ftp[:, s, :],
                                                fin[:, q + s, :], ident)
                        fts = ftsb.tile([128, 4, 128], fp16, tag="fts")
                        if copy_parity == 0:
                            nc.vector.tensor_copy(fts, ftp)
                        else:
                            nc.scalar.copy(fts, ftp)
                        copy_parity ^= 1
                        for s in range(4):
                            nc.tensor.matmul(hbank[:, q + s, :],
                                             fts[:, s, :], w1_sb,
                                             start=False,
                                             stop=(q + s == BQ - 1))
                    hrelu = hwork.tile([128, BQ, HID], fp32, tag="hrelu")
                    nc.scalar.activation(hrelu, hbank, AF.Relu)
                    hw = hwork.tile([128, BQ, HID], fp32, tag="hw")
                    nc.vector.tensor_mul(hw, hrelu, w2rep)
                    nc.vector.tensor_reduce(
                        apre[:, bq * BQ:(bq + 1) * BQ], hw,
                        axis=mybir.AxisListType.X, op=OP.add)

                # alpha/A/Bv for this t-chunk in [t, b], flip to [b, t]
                alpha = hwork.tile([128, B], fp32, tag="alpha")
                nc.scalar.activation(alpha, apre, AF.Sigmoid, bias=b2col)
                nc.vector.tensor_scalar(alpha, alpha, 0.01, 0.99,
                                        op0=OP.max, op1=OP.min)
                A_tb = hwork.tile([128, B], fp32, tag="Atb")
                nc.vector.tensor_scalar(A_tb, alpha, -1.0, 1.0,
                                        op0=OP.mult, op1=OP.add)
                Bv = hwork.tile([128, B], fp32, tag="Bv")
                nc.vector.tensor_mul(Bv, alpha, rsb[g])
                for h in range(NH):
                    aps = atps.tile([128, 2, 128], fp32, tag="aps")
                    nc.tensor.transpose(aps[:, 0, :],
                                        A_tb[:, h * 128:(h + 1) * 128],
                                        ident32)
                    nc.tensor.transpose(aps[:, 1, :],
                                        Bv[:, h * 128:(h + 1) * 128],
                                        ident32)
                    # A^T to SBUF (ACT); scans read Bv^T straight from PSUM
                    A_sb = hwork.tile([128, 128], fp32, tag="AT")
                    nc.scalar.copy(A_sb, aps[:, 0, :])
                    nc.vector.tensor_tensor_scan(
                        zpacc[:, 0, h, g * 128:(g + 1) * 128],
                        A_sb, aps[:, 1, :], 0.0,
                        op0=OP.mult, op1=OP.add)
                    nc.vector.tensor_tensor_scan(
                        zpacc[:, 1, h, g * 128:(g + 1) * 128],
                        A_sb, ones_sb, 1.0,
                        op0=OP.mult, op1=OP.mult)

            nc.sync.dma_start(zp_d[:, :, :, :], zpacc)

    nc.finalize()
    return nc


def _get_program():
    if "nc" not in _CACHE:
        _CACHE["nc"] = _build_program()
    return _CACHE["nc"]


def kernel(r, feat, W1, b1, W2, b2, _run_kwargs=None, _return_results=False):
    from concourse.bass_utils import run_bass_kernel_spmd

    r = np.asarray(r, dtype=np.float32)
    feat = np.asarray(feat, dtype=np.float32)
    W1 = np.asarray(W1, dtype=np.float32)
    b1 = np.asarray(b1, dtype=np.float32).reshape(HID)
    W2 = np.asarray(W2, dtype=np.float32).reshape(HID)
    b2 = np.asarray(b2, dtype=np.float32).reshape(1)

    nc = _get_program()
    in_maps = []
    for c in range(NCORES):
        in_maps.append({
            "feat": np.ascontiguousarray(feat[c * TLOC:(c + 1) * TLOC]),
            "r": np.ascontiguousarray(r[c * TLOC:(c + 1) * TLOC, :, 0]),
            "w1": W1, "b1": b1, "w2": W2, "b2": b2,
        })

    kw = _run_kwargs or {}
    res = run_bass_kernel_spmd(nc, in_maps, core_ids=list(range(NCORES)), **kw)

    # host stitch: per 128-t chunk, y = z + P*carry, carry chains through
    # all 32 chunks in global t order
    y = np.empty((T, B), dtype=np.float32)
    carry = r[0, :, 0].astype(np.float32)
    for c in range(NCORES):
        # zp [128 bb, 2, NH, TLOC] -> [t, b] with b = h*128 + bb
        zp = res.results[c]["zp"]
        zc = zp[:, 0].transpose(2, 1, 0).reshape(TLOC, B)
        pc = zp[:, 1].transpose(2, 1, 0).reshape(TLOC, B)
        for g in range(NTG):
            zg = zc[g * 128:(g + 1) * 128]
            pg = pc[g * 128:(g + 1) * 128]
            y_chunk = zg + pg * carry[None, :]
            carry = y_chunk[-1]
            t0 = c * TLOC + g * 128
            y[t0:t0 + 128] = y_chunk
    out = y[:, :, None]
    if _return_results:
        return out, res
    return out


# revision 38
# speedup vs baseline: 1.0456x; 1.0456x over previous
"""EMA head kernel for Trainium2 (Bass/Tile), 8 NeuronCores.

Problem: alpha = clip(sigmoid(MLP(feat)), 0.01, 0.99) per (t, b);
         y[0] = r[0]; y[t] = (1-alpha[t])*y[t-1] + alpha[t]*r[t].

Sharding: time dim T=4096 split into 8 slabs of 512 (all B=256 per core),
and each slab further into 4 chunks of 128 on-chip.  Each core computes,
per 128-t chunk, the local affine-scan pieces
    z[t] = A[t]*z[t-1] + Bv[t]   (z[-1] = 0),   A = 1-alpha, Bv = alpha*r
    P[t] = A[t]*P[t-1]           (P[-1] = 1)
and the host stitches all 32 chunks with  y = z + P * carry,
carry' = y[-1].  carry_0 = r[0] reproduces y[0] = r[0] exactly.

On-chip layout (DMA-bandwidth oriented): feat tiles keep TIME on the
partition dim: tile = feat[t0:t0+128, b0:b0+32, :] -> [128 t, 32 b, 128 f]
so each partition line is ONE contiguous 16 KB DRAM read (512 B chunks
when batch is on partitions starve the SDMA engines).  fp32->fp16 cast
happens in the SWDGE DMA.  Per b: PE transpose [t, f] -> [f, t], matmul
lhsT=[f,t] rhs=W1 -> h [t, 16] into a PSUM bank of 32 slots, then
+b1/relu/*W2/reduce -> alpha_pre [t, b], sigmoid+clip -> alpha [t, b],
Bv = alpha*r (r also loads [t, b] natively).  Per chunk, 4 small PE
transposes flip alpha/Bv to [b, t] and DVE tensor_tensor_scan runs along
the free (t=128) dim for z and P.  Param broadcasts (b1/w2/b2) are built
on-chip with K=1 outer-product matmuls -- DMA DRE replication is
pathologically slow (~7.7us per 4 KB packet) and clogs the SWDGE queue.
"""

import numpy as np

T, B, FEAT, HID = 4096, 256, 128, 16
NCORES = 8
TLOC = T // NCORES  # 512
NTG = TLOC // 128   # 4 t-chunks of 128 (the partition dim)
BQ = 32             # b-chunk per feat tile (16 KB/partition contiguous)
NBQ = B // BQ       # 8
NH = 2              # b halves of 128 for the scan layout

_CACHE = {}


def _build_program():
    import concourse.bacc as bacc
    import concourse.bass as bass
    import concourse.tile as tile
    from concourse import mybir
    from concourse.masks import make_identity

    fp32 = mybir.dt.float32
    fp16 = mybir.dt.float16
    AF = mybir.ActivationFunctionType
    OP = mybir.AluOpType

    nc = bacc.Bacc("TRN2", target_bir_lowering=False, debug=False,
                   num_devices=NCORES)

    feat_d = nc.dram_tensor("feat", [TLOC, B, FEAT], fp32, kind="ExternalInput")
    r_d = nc.dram_tensor("r", [TLOC, B], fp32, kind="ExternalInput")
    w1_d = nc.dram_tensor("w1", [FEAT, HID], fp32, kind="ExternalInput")
    b1_d = nc.dram_tensor("b1", [HID], fp32, kind="ExternalInput")
    w2_d = nc.dram_tensor("w2", [HID], fp32, kind="ExternalInput")
    b2_d = nc.dram_tensor("b2", [1], fp32, kind="ExternalInput")
    # packed z/p output: [b 128, {z,p}, half, t] -> one DMA, 4KB lines
    zp_d = nc.dram_tensor("zp", [128, 2, NH, TLOC], fp16,
                          kind="ExternalOutput")

    with tile.TileContext(nc) as tc:
        with (
            tc.tile_pool(name="singles", bufs=1) as singles,
            tc.tile_pool(name="featin", bufs=5) as featin,
            tc.tile_pool(name="ftps", bufs=4, space="PSUM") as ftps,
            tc.tile_pool(name="hps", bufs=2, space="PSUM") as hps,
            tc.tile_pool(name="atps", bufs=2, space="PSUM") as atps,
            tc.tile_pool(name="ftsb", bufs=3) as ftsb,
            tc.tile_pool(name="hwork", bufs=2) as hwork,
        ):
            # first feat tile DMAs go out before anything else queues on
            # the SWDGE ring; they stay at 1 MB so compute starts early,
            # the rest fetch 2 b-chunks (32 KB lines) per DMA
            NPEEL = 2
            fin_pre = []
            for i in range(NPEEL):
                fp = featin.tile([128, 2 * BQ, FEAT], fp16, tag="fin")
                nc.gpsimd.dma_start(fp[:, 0:BQ, :],
                                    feat_d[0:128, i * BQ:(i + 1) * BQ, :])
                fin_pre.append(fp)

            # ---- tiny param rows (keep the SWDGE queue free for feat) ----
            w1_f32 = singles.tile([128, HID], fp32)
            nc.sync.dma_start(w1_f32, w1_d[:, :])
            b1row = singles.tile([1, BQ, HID], fp16)
            nc.gpsimd.dma_start(
                b1row, bass.AP(b1_d, 0, [[0, 1], [0, BQ], [1, HID]]))
            w2row = singles.tile([1, BQ, HID], fp16)
            nc.gpsimd.dma_start(
                w2row, bass.AP(w2_d, 0, [[0, 1], [0, BQ], [1, HID]]))
            b2row = singles.tile([1, 1], fp16)
            nc.gpsimd.dma_start(b2row, bass.AP(b2_d, 0, [[0, 1], [1, 1]]))

            # r loads natively as [t, b]: contiguous 1 KB lines, no transpose
            rsb = [singles.tile([128, B], fp32, tag=f"r{g}", name=f"r{g}")
                   for g in range(NTG)]
            for g in range(NTG):
                nc.sync.dma_start(rsb[g], r_d[g * 128:(g + 1) * 128, :])

            # ---------------- constants ----------------
            ident = singles.tile([128, 128], fp16)
            make_identity(nc, ident)
            ident32 = singles.tile([128, 128], fp32)
            make_identity(nc, ident32)
            ones_col = singles.tile([1, 128], fp16)
            nc.vector.memset(ones_col, 1.0)
            ones_sb = singles.tile([128, 128], fp32)
            nc.vector.memset(ones_sb, 1.0)

            w1_sb = singles.tile([128, HID], fp16)
            nc.vector.tensor_copy(w1_sb, w1_f32)

            # broadcast w2/b2 across partitions: ones_col.T @ row
            # (b1 is injected into PSUM per tile via a K=1 matmul instead)
            w2rep = singles.tile([128, BQ, HID], fp32)
            b2col = singles.tile([128, 1], fp32)
            reps2 = atps.tile([128, BQ * HID], fp32, tag="aps")
            nc.tensor.matmul(reps2, ones_col[:, :],
                             w2row[:, :, :].rearrange("p a b -> p (a b)"))
            nc.vector.tensor_copy(
                w2rep[:, :, :].rearrange("p a b -> p (a b)"), reps2)
            reps3 = atps.tile([128, BQ * HID], fp32, tag="aps")
            nc.tensor.matmul(reps3[:, 0:1], ones_col[:, :], b2row[:, :])
            nc.vector.tensor_copy(b2col, reps3[:, 0:1])

            # packed z/P accumulator, [b, {z,p}, half, t]; one end DMA
            zpacc = singles.tile([128, 2, NH, TLOC], fp16, name="zpacc")

            copy_parity = 0
            # ---------------- main feat pipeline ----------------
            for g in range(NTG):
                apre = singles.tile([128, B], fp32, tag=f"apre{g}",
                                    name=f"apre{g}")
                for bq in range(NBQ):
                    if g == 0 and bq < NPEEL:
                        fin = fin_pre[bq][:, 0:BQ, :]
                    elif (g * NBQ + bq - NPEEL) % 2 == 0:
                        fin2 = featin.tile([128, 2 * BQ, FEAT], fp16,
                                           tag="fin")
                        nc.gpsimd.dma_start(
                            fin2, feat_d[g * 128:(g + 1) * 128,
                                         bq * BQ:(bq + 2) * BQ, :])
                        fin = fin2[:, 0:BQ, :]
                    else:
                        fin = fin2[:, BQ:2 * BQ, :]
                    hbank = hps.tile([128, BQ, HID], fp32, tag="hb")
                    # seed the whole bank with b1 (K=1 outer product);
                    # the per-slot matmuls then accumulate on top
                    nc.tensor.matmul(
                        hbank[:, :, :].rearrange("p a b -> p (a b)"),
                        ones_col[:, :],
                        b1row[:, :, :].rearrange("p a b -> p (a b)"),
                        start=True, stop=False)
                    for q in range(0, BQ, 4):
                        ftp = ftps.tile([128, 4, 128], fp16, tag="ftp")
                        for s in range(4):
                            nc.tensor.transpose(ftp[:, s, :],
                                                fin[:, q + s, :], ident)
                        fts = ftsb.tile([128, 4, 128], fp16, tag="fts")
                        if copy_parity == 0:
                            nc.vector.tensor_copy(fts, ftp)
                        else:
                            nc.scalar.copy(fts, ftp)
                        copy_parity ^= 1
                        for s in range(4):
                            nc.tensor.matmul(hbank[:, q + s, :],
                                             fts[:, s, :], w1_sb,
                                             start=False,
                                             stop=(q + s == BQ - 1))
                    hrelu = hwork.tile([128, BQ, HID], fp32, tag="hrelu")
                    nc.scalar.activation(hrelu, hbank, AF.Relu)
                    hw = hwork.tile([128, BQ, HID], fp32, tag="hw")
                    nc.vector.tensor_mul(hw, hrelu, w2rep)
                    nc.vector.tensor_reduce(
                        apre[:, bq * BQ:(bq + 1) * BQ], hw,
                        axis=mybir.AxisListType.X, op=OP.add)

                # alpha/A/Bv for this t-chunk in [t, b], flip to [b, t]
                alpha = hwork.tile([128, B], fp32, tag="alpha")
                nc.scalar.activation(alpha, apre, AF.Sigmoid, bias=b2col)
                nc.vector.tensor_scalar(alpha, alpha, 0.01, 0.99,
                                        op0=OP.max, op1=OP.min)
                A_tb = hwork.tile([128, B], fp32, tag="Atb")
                nc.vector.tensor_scalar(A_tb, alpha, -1.0, 1.0,
                                        op0=OP.mult, op1=OP.add)
                Bv = hwork.tile([128, B], fp32, tag="Bv")
                nc.vector.tensor_mul(Bv, alpha, rsb[g])
                for h in range(NH):
                    aps = atps.tile([128, 2, 128], fp32, tag="aps")
                    nc.tensor.transpose(aps[:, 0, :],
                                        A_tb[:, h * 128:(h + 1) * 128],
                                        ident32)
                    nc.tensor.transpose(aps[:, 1, :],
                                        Bv[:, h * 128:(h + 1) * 128],
                                        ident32)
                    # A^T to SBUF (ACT); scans read Bv^T straight from PSUM
                    A_sb = hwork.tile([128, 128], fp32, tag="AT")
                    nc.scalar.copy(A_sb, aps[:, 0, :])
                    nc.vector.tensor_tensor_scan(
                        zpacc[:, 0, h, g * 128:(g + 1) * 128],
                        A_sb, aps[:, 1, :], 0.0,
                        op0=OP.mult, op1=OP.add)
                    nc.vector.tensor_tensor_scan(
                        zpacc[:, 1, h, g * 128:(g + 1) * 128],
                        A_sb, ones_sb, 1.0,
                        op0=OP.mult, op1=OP.mult)

            nc.sync.dma_start(zp_d[:, :, :, :], zpacc)

    nc.finalize()
    return nc


def _get_program():
    if "nc" not in _CACHE:
        _CACHE["nc"] = _build_program()
    return _CACHE["nc"]


def kernel(r, feat, W1, b1, W2, b2, _run_kwargs=None, _return_results=False):
    from concourse.bass_utils import run_bass_kernel_spmd

    r = np.asarray(r, dtype=np.float32)
    feat = np.asarray(feat, dtype=np.float32)
    W1 = np.asarray(W1, dtype=np.float32)
    b1 = np.asarray(b1, dtype=np.float32).reshape(HID)
    W2 = np.asarray(W2, dtype=np.float32).reshape(HID)
    b2 = np.asarray(b2, dtype=np.float32).reshape(1)

    nc = _get_program()
    in_maps = []
    for c in range(NCORES):
        in_maps.append({
            "feat": np.ascontiguousarray(feat[c * TLOC:(c + 1) * TLOC]),
            "r": np.ascontiguousarray(r[c * TLOC:(c + 1) * TLOC, :, 0]),
            "w1": W1, "b1": b1, "w2": W2, "b2": b2,
        })

    kw = _run_kwargs or {}
    res = run_bass_kernel_spmd(nc, in_maps, core_ids=list(range(NCORES)), **kw)

    # host stitch: per 128-t chunk, y = z + P*carry, carry chains through
    # all 32 chunks in global t order
    y = np.empty((T, B), dtype=np.float32)
    carry = r[0, :, 0].astype(np.float32)
    for c in range(NCORES):
        # zp [128 bb, 2, NH, TLOC] -> [t, b] with b = h*128 + bb
        zp = res.results[c]["zp"]
        zc = zp[:, 0].transpose(2, 1, 0).reshape(TLOC, B)
        pc = zp[:, 1].transpose(2, 1, 0).reshape(TLOC, B)
        for g in range(NTG):
            zg = zc[g * 128:(g + 1) * 128]
            pg = pc[g * 128:(g + 1) * 128]
            y_chunk = zg + pg * carry[None, :]
            carry = y_chunk[-1]
            t0 = c * TLOC + g * 128
            y[t0:t0 + 128] = y_chunk
    out = y[:, :, None]
    if _return_results:
        return out, res
    return out


# revision 39
# speedup vs baseline: 1.0876x; 1.0401x over previous
"""EMA head kernel for Trainium2 (Bass/Tile), 8 NeuronCores.

Problem: alpha = clip(sigmoid(MLP(feat)), 0.01, 0.99) per (t, b);
         y[0] = r[0]; y[t] = (1-alpha[t])*y[t-1] + alpha[t]*r[t].

Sharding: time dim T=4096 split into 8 slabs of 512 (all B=256 per core),
and each slab further into 4 chunks of 128 on-chip.  Each core computes,
per 128-t chunk, the local affine-scan pieces
    z[t] = A[t]*z[t-1] + Bv[t]   (z[-1] = 0),   A = 1-alpha, Bv = alpha*r
    P[t] = A[t]*P[t-1]           (P[-1] = 1)
and the host stitches all 32 chunks with  y = z + P * carry,
carry' = y[-1].  carry_0 = r[0] reproduces y[0] = r[0] exactly.

On-chip layout (DMA-bandwidth oriented): feat tiles keep TIME on the
partition dim: tile = feat[t0:t0+128, b0:b0+32, :] -> [128 t, 32 b, 128 f]
so each partition line is ONE contiguous 16 KB DRAM read (512 B chunks
when batch is on partitions starve the SDMA engines).  fp32->fp16 cast
happens in the SWDGE DMA.  Per b: PE transpose [t, f] -> [f, t], matmul
lhsT=[f,t] rhs=W1 -> h [t, 16] into a PSUM bank of 32 slots, then
+b1/relu/*W2/reduce -> alpha_pre [t, b], sigmoid+clip -> alpha [t, b],
Bv = alpha*r (r also loads [t, b] natively).  Per chunk, 4 small PE
transposes flip alpha/Bv to [b, t] and DVE tensor_tensor_scan runs along
the free (t=128) dim for z and P.  Param broadcasts (b1/w2/b2) are built
on-chip with K=1 outer-product matmuls -- DMA DRE replication is
pathologically slow (~7.7us per 4 KB packet) and clogs the SWDGE queue.
"""

import numpy as np

T, B, FEAT, HID = 4096, 256, 128, 16
NCORES = 8
TLOC = T // NCORES  # 512
NTG = TLOC // 128   # 4 t-chunks of 128 (the partition dim)
BQ = 32             # b-chunk per feat tile (16 KB/partition contiguous)
NBQ = B // BQ       # 8
NH = 2              # b halves of 128 for the scan layout

_CACHE = {}


def _build_program():
    import concourse.bacc as bacc
    import concourse.bass as bass
    import concourse.tile as tile
    from concourse import mybir
    from concourse.masks import make_identity

    fp32 = mybir.dt.float32
    fp16 = mybir.dt.float16
    AF = mybir.ActivationFunctionType
    OP = mybir.AluOpType

    nc = bacc.Bacc("TRN2", target_bir_lowering=False, debug=False,
                   num_devices=NCORES)

    feat_d = nc.dram_tensor("feat", [TLOC, B, FEAT], fp32, kind="ExternalInput")
    r_d = nc.dram_tensor("r", [TLOC, B], fp32, kind="ExternalInput")
    w1_d = nc.dram_tensor("w1", [FEAT, HID], fp32, kind="ExternalInput")
    b1_d = nc.dram_tensor("b1", [HID], fp32, kind="ExternalInput")
    w2_d = nc.dram_tensor("w2", [HID], fp32, kind="ExternalInput")
    b2_d = nc.dram_tensor("b2", [1], fp32, kind="ExternalInput")
    # packed z/p output: [b 128, {z,p}, half, t] -> one DMA, 4KB lines
    zp_d = nc.dram_tensor("zp", [128, 2, NH, TLOC], fp16,
                          kind="ExternalOutput")

    with tile.TileContext(nc) as tc:
        with (
            tc.tile_pool(name="singles", bufs=1) as singles,
            tc.tile_pool(name="featin", bufs=5) as featin,
            tc.tile_pool(name="ftps", bufs=4, space="PSUM") as ftps,
            tc.tile_pool(name="hps", bufs=2, space="PSUM") as hps,
            tc.tile_pool(name="atps", bufs=2, space="PSUM") as atps,
            tc.tile_pool(name="ftsb", bufs=3) as ftsb,
            tc.tile_pool(name="hwork", bufs=2) as hwork,
        ):
            # first feat tile DMAs go out before anything else queues on
            # the SWDGE ring; they stay at 1 MB so compute starts early,
            # the rest fetch 2 b-chunks (32 KB lines) per DMA
            NPEEL = 2
            fin_pre = []
            for i in range(NPEEL):
                fp = featin.tile([128, 2 * BQ, FEAT], fp16, tag="fin")
                nc.gpsimd.dma_start(fp[:, 0:BQ, :],
                                    feat_d[0:128, i * BQ:(i + 1) * BQ, :])
                fin_pre.append(fp)

            # ---- tiny param rows via HWDGE + on-chip cast: keeps the
            # SWDGE queue 100% feat (even 4B packets interleave at
            # packet granularity and delay the stream head) ----
            w1_f32 = singles.tile([128, HID], fp32)
            nc.sync.dma_start(w1_f32, w1_d[:, :])
            b1row_f = singles.tile([1, BQ, HID], fp32)
            nc.sync.dma_start(
                b1row_f, bass.AP(b1_d, 0, [[0, 1], [0, BQ], [1, HID]]))
            w2row_f = singles.tile([1, BQ, HID], fp32)
            nc.sync.dma_start(
                w2row_f, bass.AP(w2_d, 0, [[0, 1], [0, BQ], [1, HID]]))
            b2row_f = singles.tile([1, 1], fp32)
            nc.sync.dma_start(b2row_f, bass.AP(b2_d, 0, [[0, 1], [1, 1]]))
            b1row = singles.tile([1, BQ, HID], fp16)
            nc.vector.tensor_copy(b1row, b1row_f)
            w2row = singles.tile([1, BQ, HID], fp16)
            nc.vector.tensor_copy(w2row, w2row_f)
            b2row = singles.tile([1, 1], fp16)
            nc.vector.tensor_copy(b2row, b2row_f)

            # r loads natively as [t, b]: contiguous 1 KB lines, no transpose
            rsb = [singles.tile([128, B], fp32, tag=f"r{g}", name=f"r{g}")
                   for g in range(NTG)]
            for g in range(NTG):
                nc.sync.dma_start(rsb[g], r_d[g * 128:(g + 1) * 128, :])

            # ---------------- constants ----------------
            ident = singles.tile([128, 128], fp16)
            make_identity(nc, ident)
            ident32 = singles.tile([128, 128], fp32)
            make_identity(nc, ident32)
            ones_col = singles.tile([1, 128], fp16)
            nc.vector.memset(ones_col, 1.0)
            ones_sb = singles.tile([128, 128], fp32)
            nc.vector.memset(ones_sb, 1.0)

            w1_sb = singles.tile([128, HID], fp16)
            nc.vector.tensor_copy(w1_sb, w1_f32)

            # broadcast w2/b2 across partitions: ones_col.T @ row
            # (b1 is injected into PSUM per tile via a K=1 matmul instead)
            w2rep = singles.tile([128, BQ, HID], fp32)
            b2col = singles.tile([128, 1], fp32)
            reps2 = atps.tile([128, BQ * HID], fp32, tag="aps")
            nc.tensor.matmul(reps2, ones_col[:, :],
                             w2row[:, :, :].rearrange("p a b -> p (a b)"))
            nc.vector.tensor_copy(
                w2rep[:, :, :].rearrange("p a b -> p (a b)"), reps2)
            reps3 = atps.tile([128, BQ * HID], fp32, tag="aps")
            nc.tensor.matmul(reps3[:, 0:1], ones_col[:, :], b2row[:, :])
            nc.vector.tensor_copy(b2col, reps3[:, 0:1])

            # packed z/P accumulator, [b, {z,p}, half, t]; one end DMA
            zpacc = singles.tile([128, 2, NH, TLOC], fp16, name="zpacc")

            copy_parity = 0
            # ---------------- main feat pipeline ----------------
            for g in range(NTG):
                apre = singles.tile([128, B], fp32, tag=f"apre{g}",
                                    name=f"apre{g}")
                for bq in range(NBQ):
                    if g == 0 and bq < NPEEL:
                        fin = fin_pre[bq][:, 0:BQ, :]
                    elif (g * NBQ + bq - NPEEL) % 2 == 0:
                        fin2 = featin.tile([128, 2 * BQ, FEAT], fp16,
                                           tag="fin")
                        nc.gpsimd.dma_start(
                            fin2, feat_d[g * 128:(g + 1) * 128,
                                         bq * BQ:(bq + 2) * BQ, :])
                        fin = fin2[:, 0:BQ, :]
                    else:
                        fin = fin2[:, BQ:2 * BQ, :]
                    hbank = hps.tile([128, BQ, HID], fp32, tag="hb")
                    # seed the whole bank with b1 (K=1 outer product);
                    # the per-slot matmuls then accumulate on top
                    nc.tensor.matmul(
                        hbank[:, :, :].rearrange("p a b -> p (a b)"),
                        ones_col[:, :],
                        b1row[:, :, :].rearrange("p a b -> p (a b)"),
                        start=True, stop=False)
                    for q in range(0, BQ, 4):
                        ftp = ftps.tile([128, 4, 128], fp16, tag="ftp")
                        for s in range(4):
                            nc.tensor.transpose(ftp[:, s, :],
                                                fin[:, q + s, :], ident)
                        fts = ftsb.tile([128, 4, 128], fp16, tag="fts")
                        if copy_parity == 0:
                            nc.vector.tensor_copy(fts, ftp)
                        else:
                            nc.scalar.copy(fts, ftp)
                        copy_parity ^= 1
                        for s in range(4):
                            nc.tensor.matmul(hbank[:, q + s, :],
                                             fts[:, s, :], w1_sb,
                                             start=False,
                                             stop=(q + s == BQ - 1))
                    hrelu = hwork.tile([128, BQ, HID], fp32, tag="hrelu")
                    nc.scalar.activation(hrelu, hbank, AF.Relu)
                    hw = hwork.tile([128, BQ, HID], fp32, tag="hw")
                    nc.vector.tensor_mul(hw, hrelu, w2rep)
                    nc.vector.tensor_reduce(
                        apre[:, bq * BQ:(bq + 1) * BQ], hw,
                        axis=mybir.AxisListType.X, op=OP.add)

                # alpha/A/Bv for this t-chunk in [t, b], flip to [b, t]
                alpha = hwork.tile([128, B], fp32, tag="alpha")
                nc.scalar.activation(alpha, apre, AF.Sigmoid, bias=b2col)
                nc.vector.tensor_scalar(alpha, alpha, 0.01, 0.99,
                                        op0=OP.max, op1=OP.min)
                A_tb = hwork.tile([128, B], fp32, tag="Atb")
                nc.vector.tensor_scalar(A_tb, alpha, -1.0, 1.0,
                                        op0=OP.mult, op1=OP.add)
                Bv = hwork.tile([128, B], fp32, tag="Bv")
                nc.vector.tensor_mul(Bv, alpha, rsb[g])
                for h in range(NH):
                    aps = atps.tile([128, 2, 128], fp32, tag="aps")
                    nc.tensor.transpose(aps[:, 0, :],
                                        A_tb[:, h * 128:(h + 1) * 128],
                                        ident32)
                    nc.tensor.transpose(aps[:, 1, :],
                                        Bv[:, h * 128:(h + 1) * 128],
                                        ident32)
                    # A^T to SBUF (ACT); scans read Bv^T straight from PSUM
                    A_sb = hwork.tile([128, 128], fp32, tag="AT")
                    nc.scalar.copy(A_sb, aps[:, 0, :])
                    nc.vector.tensor_tensor_scan(
                        zpacc[:, 0, h, g * 128:(g + 1) * 128],
                        A_sb, aps[:, 1, :], 0.0,
                        op0=OP.mult, op1=OP.add)
                    nc.vector.tensor_tensor_scan(
                        zpacc[:, 1, h, g * 128:(g + 1) * 128],
                        A_sb, ones_sb, 1.0,
                        op0=OP.mult, op1=OP.mult)

            nc.sync.dma_start(zp_d[:, :, :, :], zpacc)

    nc.finalize()
    return nc


def _get_program():
    if "nc" not in _CACHE:
        _CACHE["nc"] = _build_program()
    return _CACHE["nc"]


def kernel(r, feat, W1, b1, W2, b2, _run_kwargs=None, _return_results=False):
    from concourse.bass_utils import run_bass_kernel_spmd

    r = np.asarray(r, dtype=np.float32)
    feat = np.asarray(feat, dtype=np.float32)
    W1 = np.asarray(W1, dtype=np.float32)
    b1 = np.asarray(b1, dtype=np.float32).reshape(HID)
    W2 = np.asarray(W2, dtype=np.float32).reshape(HID)
    b2 = np.asarray(b2, dtype=np.float32).reshape(1)

    nc = _get_program()
    in_maps = []
    for c in range(NCORES):
        in_maps.append({
            "feat": np.ascontiguousarray(feat[c * TLOC:(c + 1) * TLOC]),
            "r": np.ascontiguousarray(r[c * TLOC:(c + 1) * TLOC, :, 0]),
            "w1": W1, "b1": b1, "w2": W2, "b2": b2,
        })

    kw = _run_kwargs or {}
    res = run_bass_kernel_spmd(nc, in_maps, core_ids=list(range(NCORES)), **kw)

    # host stitch: per 128-t chunk, y = z + P*carry, carry chains through
    # all 32 chunks in global t order
    y = np.empty((T, B), dtype=np.float32)
    carry = r[0, :, 0].astype(np.float32)
    for c in range(NCORES):
        # zp [128 bb, 2, NH, TLOC] -> [t, b] with b = h*128 + bb
        zp = res.results[c]["zp"]
        zc = zp[:, 0].transpose(2, 1, 0).reshape(TLOC, B)
        pc = zp[:, 1].transpose(2, 1, 0).reshape(TLOC, B)
        for g in range(NTG):
            zg = zc[g * 128:(g + 1) * 128]
            pg = pc[g * 128:(g + 1) * 128]
            y_chunk = zg + pg * carry[None, :]
            carry = y_chunk[-1]
            t0 = c * TLOC + g * 128
            y[t0:t0 + 128] = y_chunk
    out = y[:, :, None]
    if _return_results:
        return out, res
    return out
